# revision 24
# baseline (speedup 1.0000x reference)
"""Trainium2 Bass kernel for nn_ExpertsLinear (weighted mixture of 8 experts).

    y[b, o] = sum_e weights[b, e] * (x @ W[e] + b[e])[b, o]

Full shapes: x [65536, 512] f32, weights [65536, 8] f32,
W [8, 512, 512] f32, b [8, 1, 512] f32 -> y [65536, 512] f32.

Sharding: data-parallel over batch across 8 NeuronCores (8192 rows each);
weights replicated. Bias (always zero here) handled host-side if nonzero.

Math: gate-mean decomposition with mixed-precision experts.
  w[b,e] = m[b] + d[b,e],  m = mean_e w[b,e]
  y = m * (x @ Wsum) + sum_e d_e * (x @ W[e])
The mean channel carries ~88% of the signal and runs in fp16; the
residual expert channels have ~0.47x amplitude, so N8 of them run as
fp8(e4m3) DoubleRow matmuls (2x PE throughput) with quantization error
shrunk by the amplitude ratio. Host pre-scales: x*2^5 and W*2^16 keep
fp8 values out of the subnormal range; the 2^-21 descale is folded into
the per-sample gate tensor, so the on-device combine is identical for
all 9 channels (PSUM z times per-partition gate scalar, add tree).

Per-core kernel, per 128-row batch tile:
  - x tile SWDGE cast-DMA to fp16, SBUF->SBUF DMA transpose to
    xT [128 feat, 4, 128 b] (first HOIST tiles PE-transposed instead to
    avoid the copy-DMA/transpose-DMA xbar serialization at the head)
  - x8T = fp8(xT * 32) on gpsimd
  - 9 channels = [mean(fp16), fp16 experts..., fp8 experts...] in 3 PSUM
    groups of 3 (3 banks each, bufs=2 pool -> 6 banks, pipelined)
  - combine: group 0 scaled on ScalarE (per-channel), groups 1-2 on
    VectorE (batched broadcast mul), fp16 add tree, f32 y out
"""

import numpy as np

P = 128
D = 512
E = 8
FC = D // P
N_CORES = 8
B_FULL = 65536
B_LOC = B_FULL // N_CORES
NCH = E + 1

N8 = 6  # how many experts run in fp8 (risk/speed dial)
NF16 = E - N8  # fp16 experts; mean channel is always fp16
SX = 32.0  # x pre-scale for fp8 (max |x*32| ~ 173 < 240 e4m3 max normal)
SW = 32768.0  # W pre-scale for fp8 (max |W*2^15| ~ 157 < 240)
DEBUG_XQ = False  # emit the device-quantized x tile 0 for rounding checks

_COMPILED = {}


def _build_nc(b_loc=B_LOC, num_devices=N_CORES, debug_xq=None):
    import concourse.bacc as bacc
    import concourse.mybir as mybir
    import concourse.tile as tile
    from concourse.masks import make_identity

    if debug_xq is None:
        debug_xq = DEBUG_XQ

    F32 = mybir.dt.float32
    F16 = mybir.dt.float16
    F8 = mybir.dt.float8e4
    DR = mybir.MatmulPerfMode.DoubleRow

    nc = bacc.Bacc(
        "TRN2",
        target_bir_lowering=False,
        debug=False,
        enable_asserts=False,
        num_devices=num_devices,
    )
    x_d = nc.dram_tensor("x", [b_loc, D], F32, kind="ExternalInput").ap()
    g_d = nc.dram_tensor("g", [b_loc, NCH], F32, kind="ExternalInput").ap()
    # fp16 weight slots: [Wsum, fp16 experts...]; fp8 slots: remaining experts
    Wf_d = nc.dram_tensor("Wf16", [NF16 + 1, D, D], F16, kind="ExternalInput").ap()
    W8_d = nc.dram_tensor("W8", [max(N8, 1), D, D], F8, kind="ExternalInput").ap()
    y_d = nc.dram_tensor("y", [b_loc, D], F32, kind="ExternalOutput").ap()
    if debug_xq:
        xq_d = nc.dram_tensor("xq", [P, FC, P], F8, kind="ExternalOutput").ap()

    nbt = b_loc // P
    HOIST = 3

    # channel table: (kind, slot); gathered into 5 groups of <=2 so PSUM
    # tiles are 2 banks each and a bufs=4 pool gives ~3 groups of slack
    channels = [("f16", s) for s in range(NF16 + 1)] + [("f8", j) for j in range(N8)]
    groups = [channels[i : i + 2] for i in range(0, NCH, 2)]

    with tile.TileContext(nc) as tc:
        with (
            tc.tile_pool(name="const", bufs=1) as const_pool,
            tc.tile_pool(name="xf32", bufs=3) as xf_pool,
            tc.tile_pool(name="xh16", bufs=3) as xh_pool,
            tc.tile_pool(name="xT16", bufs=3) as xT_pool,
            tc.tile_pool(name="x8T", bufs=3) as x8_pool,
            tc.tile_pool(name="tmul", bufs=8) as t_pool,
            tc.tile_pool(name="yout", bufs=3) as y_pool,
        ):
            def load_x(bt):
                xh = xh_pool.tile([P, D], F16, name="xh", tag="xh")
                nc.gpsimd.dma_start(out=xh[:], in_=x_d[bt * P : (bt + 1) * P, :])
                xT = xT_pool.tile([P, FC, P], F16, name="xT", tag="xT")
                nc.sync.dma_start_transpose(xT[:], xh[:])
                return xT

            ident = const_pool.tile([P, P], F16, name="ident")
            make_identity(nc, ident)

            head_xh = []
            for bt in range(min(HOIST, nbt)):
                xf = xf_pool.tile([P, D], F32, name="xf", tag="xf")
                nc.sync.dma_start(out=xf[:], in_=x_d[bt * P : (bt + 1) * P, :])
                xh = xh_pool.tile([P, D], F16, name="xh", tag="xh")
                nc.vector.tensor_copy(out=xh[:], in_=xf[:])
                head_xh.append(xh)

            # Resident gates first (tiny; the first combines need them before
            # the 3MB of weights finishes): g_sb[p, t, c] = g[t*128+p, c]
            g_sb = const_pool.tile([P, nbt, NCH], F32, name="g_sb")
            nc.sync.dma_start(out=g_sb[:], in_=g_d.rearrange("(t p) c -> p t c", p=P))

            # Resident weights, split across both HWDGE queues
            Wf_sb = const_pool.tile([P, NF16 + 1, FC, D], F16, name="Wf_sb")
            for s in range(NF16 + 1):
                eng = nc.sync if s % 2 == 0 else nc.scalar
                eng.dma_start(
                    out=Wf_sb[:, s], in_=Wf_d[s].rearrange("(fc p) o -> p fc o", p=P)
                )
            W8_sb = const_pool.tile([P, max(N8, 1), FC, D], F8, name="W8_sb")
            for j in range(N8):
                eng = nc.sync if j % 2 == 1 else nc.scalar
                eng.dma_start(
                    out=W8_sb[:, j], in_=W8_d[j].rearrange("(fc p) o -> p fc o", p=P)
                )

            xT_pending = {}
            with tc.tile_pool(name="tph", bufs=2, space="PSUM") as tp_pool:
                for bt in range(min(HOIST, nbt)):
                    tp = tp_pool.tile([P, FC, P], F16, name="tp", tag="tp")
                    for fc in range(FC):
                        nc.tensor.transpose(
                            tp[:, fc, :],
                            head_xh[bt][:, fc * P : (fc + 1) * P],
                            ident[:],
                        )
                    xT = xT_pool.tile([P, FC, P], F16, name="xT", tag="xT")
                    nc.vector.tensor_copy(out=xT[:], in_=tp[:])
                    xT_pending[bt] = xT

            z_pool = tc.alloc_tile_pool(name="zpsum", bufs=4, space="PSUM")
            for bt in range(nbt):
                xT = xT_pending.pop(bt) if bt in xT_pending else load_x(bt)
                # fp8 quant on ScalarE: ACT has slack and, unlike DVE, its
                # queue reaches this op before the PE needs the fp8 lhsT
                x8T = x8_pool.tile([P, FC, P], F8, name="x8T", tag="x8T")
                nc.scalar.mul(x8T[:], xT[:], SX)
                if debug_xq and bt == 0:
                    nc.sync.dma_start(out=xq_d, in_=x8T[:])

                zts = []
                for gi in range(len(groups)):
                    zt = z_pool.tile([P, 2, D], F32, name="zt", tag="zt")
                    for ci, (kind, slot) in enumerate(groups[gi]):
                        if kind == "f16":
                            for fc in range(FC):
                                nc.tensor.matmul(
                                    zt[:, ci, :],
                                    lhsT=xT[:, fc, :],
                                    rhs=Wf_sb[:, slot, fc, :],
                                    start=(fc == 0),
                                    stop=(fc == FC - 1),
                                )
                        else:
                            for jp in range(FC // 2):
                                nc.tensor.matmul(
                                    zt[:, ci, :],
                                    lhsT=x8T[:, 2 * jp : 2 * jp + 2, :],
                                    rhs=W8_sb[:, slot, 2 * jp : 2 * jp + 2, :],
                                    start=(jp == 0),
                                    stop=(jp == FC // 2 - 1),
                                    perf_mode=DR,
                                )
                    zts.append(zt)

                # Combine: y = sum_c g[:, c] * z_c. PSUM-releasing reads run
                # as each group completes: groups 0,2,4 on ScalarE
                # (per-channel muls), groups 1,3 on VectorE (batched mul).
                tP = []
                for gi in range(len(groups)):
                    n = len(groups[gi])
                    t = t_pool.tile([P, n, D], F16, name=f"tP{gi}", tag=f"tP{gi}")
                    lo = 2 * gi
                    if gi % 2 == 0:
                        for ci in range(n):
                            nc.scalar.mul(
                                t[:, ci, :],
                                zts[gi][:, ci, :],
                                g_sb[:, bt, lo + ci : lo + ci + 1],
                            )
                    else:
                        wB = g_sb[:, bt, lo : lo + n, None].to_broadcast([P, n, D])
                        nc.vector.tensor_mul(out=t[:], in0=zts[gi][:, 0:n, :], in1=wB)
                    tP.append(t)

                # add tree: pair sums on VectorE, channel-fold tail on GpSimd
                a = t_pool.tile([P, 2, D], F16, name="a", tag="a")
                nc.vector.tensor_add(out=a[:], in0=tP[0][:], in1=tP[1][:])
                b = t_pool.tile([P, 2, D], F16, name="b", tag="b")
                nc.gpsimd.tensor_add(out=b[:], in0=tP[2][:], in1=tP[3][:])
                c = t_pool.tile([P, 2, D], F16, name="c", tag="c")
                nc.vector.tensor_add(out=c[:], in0=a[:], in1=b[:])
                uv = t_pool.tile([P, D], F16, name="uv", tag="uv")
                nc.gpsimd.tensor_add(out=uv[:], in0=c[:, 0, :], in1=c[:, 1, :])
                y_t = y_pool.tile([P, D], F32, name="y_t")
                nc.gpsimd.tensor_add(out=y_t[:], in0=uv[:], in1=tP[4][:, 0, :])

                # plain f32 store on the sync HWDGE queue (keeps the SWDGE
                # queue cast-only for x loads; avoids extra xbar mode thrash)
                nc.sync.dma_start(out=y_d[bt * P : (bt + 1) * P, :], in_=y_t[:])

            z_pool.release()

    nc.compile()
    return nc


def _get_nc():
    if "nc" not in _COMPILED:
        _COMPILED["nc"] = _build_nc()
    return _COMPILED["nc"]


def make_in_maps(x, weights, W):
    """Host-side prep: shard x/gates, build weight tensors + gate tensor."""
    import ml_dtypes

    x = np.ascontiguousarray(np.asarray(x, dtype=np.float32))
    w = np.asarray(weights, dtype=np.float32)
    W = np.asarray(W, dtype=np.float32)

    m = w.mean(axis=1, keepdims=True)
    d = w - m
    g = np.empty((B_FULL, NCH), dtype=np.float32)
    g[:, 0:1] = m
    for i in range(NF16):
        g[:, 1 + i] = d[:, i]
    for j in range(N8):
        g[:, 1 + NF16 + j] = d[:, NF16 + j] / (SX * SW)
    g = np.ascontiguousarray(g)

    Wf = np.empty((NF16 + 1, D, D), dtype=np.float16)
    Wf[0] = W.sum(axis=0).astype(np.float16)
    for i in range(NF16):
        Wf[1 + i] = W[i].astype(np.float16)
    # NB: device float8e4 is IEEE e4m3 (max normal 240, exp=1111 -> inf/nan),
    # NOT e4m3fn. Values must stay under 240 after scaling.
    W8 = np.ascontiguousarray(
        (W[NF16:] * SW).astype(ml_dtypes.float8_e4m3)
    ) if N8 else np.zeros((1, D, D), dtype=ml_dtypes.float8_e4m3)

    xs = x.reshape(N_CORES, B_LOC, D)
    gs = g.reshape(N_CORES, B_LOC, NCH)
    return [
        {"x": xs[c], "g": gs[c], "Wf16": Wf, "W8": W8} for c in range(N_CORES)
    ]


def kernel(x, weights, W, b):
    from concourse.bass_utils import run_bass_kernel_spmd

    b_np = np.asarray(b, dtype=np.float32)
    nc = _get_nc()
    in_maps = make_in_maps(x, weights, W)
    res = run_bass_kernel_spmd(nc, in_maps, core_ids=list(range(N_CORES)))
    y = np.concatenate([res.results[c]["y"] for c in range(N_CORES)], axis=0)

    if np.any(b_np):
        y = y + np.asarray(weights, np.float32) @ b_np[:, 0, :]

    return y.astype(np.float32)


# revision 26
# speedup vs baseline: 1.0439x; 1.0439x over previous
"""Trainium2 Bass kernel for nn_ExpertsLinear (weighted mixture of 8 experts).

    y[b, o] = sum_e weights[b, e] * (x @ W[e] + b[e])[b, o]

Full shapes: x [65536, 512] f32, weights [65536, 8] f32,
W [8, 512, 512] f32, b [8, 1, 512] f32 -> y [65536, 512] f32.

Sharding: data-parallel over batch across 8 NeuronCores (8192 rows each);
weights replicated. Bias (always zero here) handled host-side if nonzero.

Math: gate-mean decomposition with mixed-precision experts.
  w[b,e] = m[b] + d[b,e],  m = mean_e w[b,e]
  y = m * (x @ Wsum) + sum_e d_e * (x @ W[e])
The mean channel carries ~88% of the signal and runs in fp16; the
residual expert channels have ~0.47x amplitude, so N8 of them run as
fp8(e4m3) DoubleRow matmuls (2x PE throughput) with quantization error
shrunk by the amplitude ratio. Host pre-scales: x*2^5 and W*2^16 keep
fp8 values out of the subnormal range; the 2^-21 descale is folded into
the per-sample gate tensor, so the on-device combine is identical for
all 9 channels (PSUM z times per-partition gate scalar, add tree).

Per-core kernel, per 128-row batch tile:
  - x tile SWDGE cast-DMA to fp16, SBUF->SBUF DMA transpose to
    xT [128 feat, 4, 128 b] (first HOIST tiles PE-transposed instead to
    avoid the copy-DMA/transpose-DMA xbar serialization at the head)
  - x8T = fp8(xT * 32) on gpsimd
  - 9 channels = [mean(fp16), fp16 experts..., fp8 experts...] in 3 PSUM
    groups of 3 (3 banks each, bufs=2 pool -> 6 banks, pipelined)
  - combine: group 0 scaled on ScalarE (per-channel), groups 1-2 on
    VectorE (batched broadcast mul), fp16 add tree, f32 y out
"""

import numpy as np

P = 128
D = 512
E = 8
FC = D // P
N_CORES = 8
B_FULL = 65536
B_LOC = B_FULL // N_CORES
NCH = E + 1

N8 = 6  # how many experts run in fp8 (risk/speed dial)
NF16 = E - N8  # fp16 experts; mean channel is always fp16
SX = 32.0  # x pre-scale for fp8 (max |x*32| ~ 173 < 240 e4m3 max normal)
SW = 32768.0  # W pre-scale for fp8 (max |W*2^15| ~ 157 < 240)
DEBUG_XQ = False  # emit the device-quantized x tile 0 for rounding checks

_COMPILED = {}


def _build_nc(b_loc=B_LOC, num_devices=N_CORES, debug_xq=None):
    import concourse.bacc as bacc
    import concourse.mybir as mybir
    import concourse.tile as tile
    from concourse.masks import make_identity

    if debug_xq is None:
        debug_xq = DEBUG_XQ

    F32 = mybir.dt.float32
    F16 = mybir.dt.float16
    F8 = mybir.dt.float8e4
    DR = mybir.MatmulPerfMode.DoubleRow

    nc = bacc.Bacc(
        "TRN2",
        target_bir_lowering=False,
        debug=False,
        enable_asserts=False,
        num_devices=num_devices,
    )
    x_d = nc.dram_tensor("x", [b_loc, D], F32, kind="ExternalInput").ap()
    g_d = nc.dram_tensor("g", [b_loc, NCH], F32, kind="ExternalInput").ap()
    # fp16 weight slots: [Wsum, fp16 experts...]; fp8 slots: remaining experts
    Wf_d = nc.dram_tensor("Wf16", [NF16 + 1, D, D], F16, kind="ExternalInput").ap()
    W8_d = nc.dram_tensor("W8", [max(N8, 1), D, D], F8, kind="ExternalInput").ap()
    y_d = nc.dram_tensor("y", [b_loc, D], F32, kind="ExternalOutput").ap()
    if debug_xq:
        xq_d = nc.dram_tensor("xq", [P, FC, P], F8, kind="ExternalOutput").ap()

    nbt = b_loc // P
    HOIST = 3

    # channel table: (kind, slot); gathered into 5 groups of <=2 so PSUM
    # tiles are 2 banks each and a bufs=4 pool gives ~3 groups of slack
    channels = [("f16", s) for s in range(NF16 + 1)] + [("f8", j) for j in range(N8)]
    groups = [channels[i : i + 2] for i in range(0, NCH, 2)]

    with tile.TileContext(nc) as tc:
        with (
            tc.tile_pool(name="const", bufs=1) as const_pool,
            tc.tile_pool(name="xf32", bufs=3) as xf_pool,
            tc.tile_pool(name="xh16", bufs=3) as xh_pool,
            tc.tile_pool(name="xT16", bufs=3) as xT_pool,
            tc.tile_pool(name="x8T", bufs=3) as x8_pool,
            tc.tile_pool(name="tmul", bufs=8) as t_pool,
            tc.tile_pool(name="yout", bufs=3) as y_pool,
        ):
            def load_x(bt):
                xh = xh_pool.tile([P, D], F16, name="xh", tag="xh")
                nc.gpsimd.dma_start(out=xh[:], in_=x_d[bt * P : (bt + 1) * P, :])
                xT = xT_pool.tile([P, FC, P], F16, name="xT", tag="xT")
                nc.sync.dma_start_transpose(xT[:], xh[:])
                return xT

            ident = const_pool.tile([P, P], F16, name="ident")
            make_identity(nc, ident)

            head_xh = []
            for bt in range(min(HOIST, nbt)):
                xf = xf_pool.tile([P, D], F32, name="xf", tag="xf")
                nc.sync.dma_start(out=xf[:], in_=x_d[bt * P : (bt + 1) * P, :])
                xh = xh_pool.tile([P, D], F16, name="xh", tag="xh")
                nc.vector.tensor_copy(out=xh[:], in_=xf[:])
                head_xh.append(xh)

            # Resident gates first (tiny; the first combines need them before
            # the 3MB of weights finishes): g_sb[p, t, c] = g[t*128+p, c]
            g_sb = const_pool.tile([P, nbt, NCH], F32, name="g_sb")
            nc.sync.dma_start(out=g_sb[:], in_=g_d.rearrange("(t p) c -> p t c", p=P))

            # Resident weights, split across both HWDGE queues
            Wf_sb = const_pool.tile([P, NF16 + 1, FC, D], F16, name="Wf_sb")
            for s in range(NF16 + 1):
                eng = nc.sync if s % 2 == 0 else nc.scalar
                eng.dma_start(
                    out=Wf_sb[:, s], in_=Wf_d[s].rearrange("(fc p) o -> p fc o", p=P)
                )
            W8_sb = const_pool.tile([P, max(N8, 1), FC, D], F8, name="W8_sb")
            for j in range(N8):
                eng = nc.sync if j % 2 == 1 else nc.scalar
                eng.dma_start(
                    out=W8_sb[:, j], in_=W8_d[j].rearrange("(fc p) o -> p fc o", p=P)
                )

            xT_pending = {}
            with tc.tile_pool(name="tph", bufs=2, space="PSUM") as tp_pool:
                for bt in range(min(HOIST, nbt)):
                    tp = tp_pool.tile([P, FC, P], F16, name="tp", tag="tp")
                    for fc in range(FC):
                        nc.tensor.transpose(
                            tp[:, fc, :],
                            head_xh[bt][:, fc * P : (fc + 1) * P],
                            ident[:],
                        )
                    xT = xT_pool.tile([P, FC, P], F16, name="xT", tag="xT")
                    nc.vector.tensor_copy(out=xT[:], in_=tp[:])
                    xT_pending[bt] = xT

            z_pool = tc.alloc_tile_pool(name="zpsum", bufs=4, space="PSUM")
            for bt in range(nbt):
                xT = xT_pending.pop(bt) if bt in xT_pending else load_x(bt)
                x8T = x8_pool.tile([P, FC, P], F8, name="x8T", tag="x8T")
                nc.vector.tensor_scalar_mul(x8T[:], xT[:], SX)
                if debug_xq and bt == 0:
                    nc.sync.dma_start(out=xq_d, in_=x8T[:])

                zts = []
                for gi in range(len(groups)):
                    zt = z_pool.tile([P, 2, D], F32, name="zt", tag="zt")
                    for ci, (kind, slot) in enumerate(groups[gi]):
                        if kind == "f16":
                            for fc in range(FC):
                                nc.tensor.matmul(
                                    zt[:, ci, :],
                                    lhsT=xT[:, fc, :],
                                    rhs=Wf_sb[:, slot, fc, :],
                                    start=(fc == 0),
                                    stop=(fc == FC - 1),
                                )
                        else:
                            for jp in range(FC // 2):
                                nc.tensor.matmul(
                                    zt[:, ci, :],
                                    lhsT=x8T[:, 2 * jp : 2 * jp + 2, :],
                                    rhs=W8_sb[:, slot, 2 * jp : 2 * jp + 2, :],
                                    start=(jp == 0),
                                    stop=(jp == FC // 2 - 1),
                                    perf_mode=DR,
                                )
                    zts.append(zt)

                # Combine: y = sum_c g[:, c] * z_c. PSUM-releasing reads run
                # as each group completes: groups 0,2,4 on ScalarE
                # (per-channel muls), groups 1,3 on VectorE (batched mul).
                tP = []
                for gi in range(len(groups)):
                    n = len(groups[gi])
                    t = t_pool.tile([P, n, D], F16, name=f"tP{gi}", tag=f"tP{gi}")
                    lo = 2 * gi
                    if gi % 2 == 0:
                        for ci in range(n):
                            nc.scalar.mul(
                                t[:, ci, :],
                                zts[gi][:, ci, :],
                                g_sb[:, bt, lo + ci : lo + ci + 1],
                            )
                    else:
                        wB = g_sb[:, bt, lo : lo + n, None].to_broadcast([P, n, D])
                        nc.vector.tensor_mul(out=t[:], in0=zts[gi][:, 0:n, :], in1=wB)
                    tP.append(t)

                # add tree: pair sums on VectorE, channel-fold tail on GpSimd
                a = t_pool.tile([P, 2, D], F16, name="a", tag="a")
                nc.vector.tensor_add(out=a[:], in0=tP[0][:], in1=tP[1][:])
                b = t_pool.tile([P, 2, D], F16, name="b", tag="b")
                nc.vector.tensor_add(out=b[:], in0=tP[2][:], in1=tP[3][:])
                c = t_pool.tile([P, 2, D], F16, name="c", tag="c")
                nc.vector.tensor_add(out=c[:], in0=a[:], in1=b[:])
                uv = t_pool.tile([P, D], F16, name="uv", tag="uv")
                nc.gpsimd.tensor_add(out=uv[:], in0=c[:, 0, :], in1=c[:, 1, :])
                y_t = y_pool.tile([P, D], F32, name="y_t")
                nc.gpsimd.tensor_add(out=y_t[:], in0=uv[:], in1=tP[4][:, 0, :])

                # plain f32 store on the sync HWDGE queue (keeps the SWDGE
                # queue cast-only for x loads; avoids extra xbar mode thrash)
                nc.sync.dma_start(out=y_d[bt * P : (bt + 1) * P, :], in_=y_t[:])

            z_pool.release()

    nc.compile()
    return nc


def _get_nc():
    if "nc" not in _COMPILED:
        _COMPILED["nc"] = _build_nc()
    return _COMPILED["nc"]


def make_in_maps(x, weights, W):
    """Host-side prep: shard x/gates, build weight tensors + gate tensor."""
    import ml_dtypes

    x = np.ascontiguousarray(np.asarray(x, dtype=np.float32))
    w = np.asarray(weights, dtype=np.float32)
    W = np.asarray(W, dtype=np.float32)

    m = w.mean(axis=1, keepdims=True)
    d = w - m
    g = np.empty((B_FULL, NCH), dtype=np.float32)
    g[:, 0:1] = m
    for i in range(NF16):
        g[:, 1 + i] = d[:, i]
    for j in range(N8):
        g[:, 1 + NF16 + j] = d[:, NF16 + j] / (SX * SW)
    g = np.ascontiguousarray(g)

    Wf = np.empty((NF16 + 1, D, D), dtype=np.float16)
    Wf[0] = W.sum(axis=0).astype(np.float16)
    for i in range(NF16):
        Wf[1 + i] = W[i].astype(np.float16)
    # NB: device float8e4 is IEEE e4m3 (max normal 240, exp=1111 -> inf/nan),
    # NOT e4m3fn. Values must stay under 240 after scaling.
    W8 = np.ascontiguousarray(
        (W[NF16:] * SW).astype(ml_dtypes.float8_e4m3)
    ) if N8 else np.zeros((1, D, D), dtype=ml_dtypes.float8_e4m3)

    xs = x.reshape(N_CORES, B_LOC, D)
    gs = g.reshape(N_CORES, B_LOC, NCH)
    return [
        {"x": xs[c], "g": gs[c], "Wf16": Wf, "W8": W8} for c in range(N_CORES)
    ]


def kernel(x, weights, W, b):
    from concourse.bass_utils import run_bass_kernel_spmd

    b_np = np.asarray(b, dtype=np.float32)
    nc = _get_nc()
    in_maps = make_in_maps(x, weights, W)
    res = run_bass_kernel_spmd(nc, in_maps, core_ids=list(range(N_CORES)))
    y = np.concatenate([res.results[c]["y"] for c in range(N_CORES)], axis=0)

    if np.any(b_np):
        y = y + np.asarray(weights, np.float32) @ b_np[:, 0, :]

    return y.astype(np.float32)
